# revision 8
# baseline (speedup 1.0000x reference)
"""GQA attention block (B=1, T=2048, HID=2048, NQ=16, NKV=8, D=128) on 8 TRN2
NeuronCores.

Sharding: tensor-parallel over heads. Core c owns q-heads {2c, 2c+1} and
kv-head c. The 8 partial outputs are summed on the host (scaled 1/(VS*BETA)).

v2 speed strategy (validated against the TimelineSim cost model + f64 ref):
  - projections: 3-term split-fp8 (xh*wh + xl*wh + xh*wl) with K=256
    DoubleRow matmuls (0.5 cyc/row in the cost model). Wq/Wk pre-scaled by
    WS=64 (cancels through RMS norm), Wv by VS=32.
  - V is projected directly transposed ([t, d] tiles, stationary = x chunk)
    so no PE transposes / identity are needed.
  - q/k: RMS-norm + RoPE on DVE in bf16; Act reads raw projections straight
    from PSUM (no staging copy).
  - attention: at = exp(score/sqrt(D) - 2) fp8 for q-rows >= 512, bf16 for
    the first 512 rows. Causal handling is fine-grained on the diagonal
    512x512 block: per 128-query subtile only the needed key tiles are
    computed, and only the true-diagonal 128x128 tile is min-masked
    (mask in {0, 240}: min(sat, 0) = 0 kills acausal fp8-overflowed exp).
  - denominators: ones-stationary matmuls accumulated alongside ctx (PSUM).
  - o_proj: 3-term split-fp8 DoubleRow with BOTH heads packed into K=256
    (ctx split hi/lo on DVE/Pool; Wo pre-scaled by BETA=64, ctx carries
    VS=32; host divides by 2048). Output rows for q-blocks {0,2,3} are
    DMA'd f32 straight from PSUM (no copy); the last-processed block (1)
    goes through bf16 copies for a short tail.
  - schedule: q-blocks processed in order 0,3,2,1; each block's o_proj
    tiles are interleaved as PE filler into the next block's attention
    (which is Act-exp paced), keeping the PE queue dense.
"""

import sys

sys.path.insert(0, "/opt/trn_rl_repo")

import numpy as np
import ml_dtypes

import concourse.bass as bass  # noqa: F401  (bass must import before tile)
import concourse.mybir as mybir
import concourse.tile as tile
from concourse import bacc
from concourse.bass_utils import run_bass_kernel_spmd

N_CORES = 8
T = 2048
HID = 2048
NQ, NKV, D = 16, 8, 128
HQ = NQ // N_CORES  # q heads per core = 2
EPS = 1e-6
SCALE = D**-0.5
SHIFT = 2.0
WS = 64.0   # Wq/Wk pre-scale (cancels in RMS norm)
VS = 32.0   # Wv pre-scale == ctx scale alpha (fp8 range)
BETA = 64.0  # Wo pre-scale (fp8 range); host divides by VS*BETA

P = 128
H = D // 2
KP = HID // 256     # 8 K-pair chunks of 256
NTR = T // 512      # 4 T-ranges of 512

F32 = mybir.dt.float32
BF16 = mybir.dt.bfloat16
F8 = mybir.dt.float8e4
DR = mybir.MatmulPerfMode.DoubleRow
ACT_EXP = mybir.ActivationFunctionType.Exp
ACT_SQRT = mybir.ActivationFunctionType.Sqrt
ACT_SQUARE = mybir.ActivationFunctionType.Square
MIN = mybir.AluOpType.min
MULT = mybir.AluOpType.mult
SUB = mybir.AluOpType.subtract

QR_ORDER = [0, 3, 2, 1]  # last one takes the bf16-copy output path


def build_nc():
    nc = bacc.Bacc("TRN2", target_bir_lowering=False, debug=False,
                   num_devices=N_CORES)

    # ---- DRAM tensors (names = in_map keys) ----
    xh = nc.dram_tensor("xh", [P, KP, 2, T], F8, kind="ExternalInput")
    xl = nc.dram_tensor("xl", [P, KP, 2, T], F8, kind="ExternalInput")
    wqh = nc.dram_tensor("wqh", [P, KP, 2, HQ * D], F8, kind="ExternalInput")
    wql = nc.dram_tensor("wql", [P, KP, 2, HQ * D], F8, kind="ExternalInput")
    wkh = nc.dram_tensor("wkh", [P, KP, 2, D], F8, kind="ExternalInput")
    wkl = nc.dram_tensor("wkl", [P, KP, 2, D], F8, kind="ExternalInput")
    wvh = nc.dram_tensor("wvh", [P, KP, 2, D], F8, kind="ExternalInput")
    wvl = nc.dram_tensor("wvl", [P, KP, 2, D], F8, kind="ExternalInput")
    woh = nc.dram_tensor("woh", [P, HQ, HID], F8, kind="ExternalInput")
    wol = nc.dram_tensor("wol", [P, HQ, HID], F8, kind="ExternalInput")
    cosT = nc.dram_tensor("cosT", [P, T], BF16, kind="ExternalInput")
    sinT = nc.dram_tensor("sinT", [P, T], BF16, kind="ExternalInput")
    qw = nc.dram_tensor("qw", [P, 1], F32, kind="ExternalInput")
    kw = nc.dram_tensor("kw", [P, 1], F32, kind="ExternalInput")
    masks = nc.dram_tensor("masks", [P, P], BF16, kind="ExternalInput")
    out = nc.dram_tensor("out", [T, HID], BF16, kind="ExternalOutput")

    with tile.TileContext(nc) as tc:
        with (
            tc.tile_pool(name="cst", bufs=1) as cst,
            tc.tile_pool(name="fin", bufs=1) as fin,
        ):
            # ---------- constants / weights resident in SBUF ----------
            wqh_sb = cst.tile([P, KP, 2, HQ * D], F8)
            wql_sb = cst.tile([P, KP, 2, HQ * D], F8)
            wkh_sb = cst.tile([P, KP, 2, D], F8)
            wkl_sb = cst.tile([P, KP, 2, D], F8)
            wvh_sb = cst.tile([P, KP, 2, D], F8)
            wvl_sb = cst.tile([P, KP, 2, D], F8)
            woh_sb = cst.tile([P, HQ, HID], F8)
            wol_sb = cst.tile([P, HQ, HID], F8)
            masks_sb = cst.tile([P, P], BF16)
            cos_sb = cst.tile([P, T], BF16)
            sin_sb = cst.tile([P, T], BF16)
            qw_sb = cst.tile([P, 1], F32)
            kw_sb = cst.tile([P, 1], F32)
            nc.gpsimd.dma_start(qw_sb[:], qw[:])
            nc.gpsimd.dma_start(kw_sb[:], kw[:])
            ones_b = cst.tile([P, 1], BF16)
            nc.vector.memset(ones_b[:], 1.0)
            # DoubleRow ldweights requires the 2-plane dim step % 16 == 0
            w1_8 = cst.tile([P, 2, 16], F8)
            nc.vector.memset(w1_8[:], 1.0)
            eps_sb = cst.tile([1, 1], F32)
            nc.vector.memset(eps_sb[:], EPS * WS * WS)
            shift_sb = cst.tile([P, 1], F32)
            nc.vector.memset(shift_sb[:], -SHIFT)

            # post RMS+RoPE q/k in bf16 (d on partitions)
            qT = [fin.tile([P, T], BF16, name=f"qT_{s}") for s in range(3)]
            # V (VS x): fp8 plane-pairs (plane = st parity) + bf16 st 0-3
            vp = fin.tile([P, T // 256, 2, D], F8)
            v0b = fin.tile([P, 4, D], BF16)
            # normalized ctx (VS x), fp8 hi/lo, plane = head
            ctxC = fin.tile([P, HQ, T], F8)
            ctxL = fin.tile([P, HQ, T], F8)

            # ==== Phase A (split-fp8 DR projections) + B (RMS+RoPE) ====
            with (
                tc.tile_pool(name="xp", bufs=4) as xp,
                tc.tile_pool(name="tmpp", bufs=6) as tmpp,
                tc.tile_pool(name="psA", bufs=4, space="PSUM") as psA,
                tc.tile_pool(name="psV", bufs=2, space="PSUM") as psV,
                tc.tile_pool(name="psB", bufs=2, space="PSUM") as psB,
            ):
                for tr in range(NTR):
                    ts = slice(tr * 512, (tr + 1) * 512)
                    xhc = xp.tile([P, KP, 2, 512], F8, name="xhc")
                    xlc = xp.tile([P, KP, 2, 512], F8, name="xlc")
                    if tr == 0:
                        # ordered for fastest PE start: wq-hi, x-hi first
                        nc.sync.dma_start(wqh_sb[:], wqh[:])
                        nc.scalar.dma_start(xhc[:, 0:4, :, :], xh[:, 0:4, :, ts])
                        nc.sync.dma_start(wkh_sb[:], wkh[:])
                        nc.sync.dma_start(wvh_sb[:], wvh[:])
                        nc.scalar.dma_start(xhc[:, 4:8, :, :], xh[:, 4:8, :, ts])
                        nc.sync.dma_start(xlc[:], xl[:, :, :, ts])
                        nc.sync.dma_start(wql_sb[:], wql[:])
                        nc.sync.dma_start(wkl_sb[:], wkl[:])
                        nc.sync.dma_start(wvl_sb[:], wvl[:])
                        nc.scalar.dma_start(cos_sb[:], cosT[:])
                        nc.scalar.dma_start(sin_sb[:], sinT[:])
                    else:
                        nc.sync.dma_start(xhc[:], xh[:, :, :, ts])
                        nc.sync.dma_start(xlc[:], xl[:, :, :, ts])
                    if tr == 1:
                        nc.scalar.dma_start(masks_sb[:], masks[:])
                        nc.gpsimd.dma_start(woh_sb[:], woh[:])
                        nc.gpsimd.dma_start(wol_sb[:], wol[:])

                    terms = ((wqh_sb, wkh_sb, wvh_sb, xhc),
                             (wqh_sb, wkh_sb, wvh_sb, xlc),
                             (wql_sb, wkl_sb, wvl_sb, xhc))

                    # --- projections q0, q1, k (d on partitions) ---
                    raw = []
                    for s in range(3):
                        ps = psA.tile([P, 512], F32, name="psA_t")
                        n = 3 * KP
                        i = 0
                        for wq_t, wk_t, _, xt_ in terms:
                            wt = wq_t if s < 2 else wk_t
                            cs = slice(s * D, (s + 1) * D) if s < 2 \
                                else slice(0, D)
                            for kp in range(KP):
                                nc.tensor.matmul(
                                    ps[:], wt[:, kp, :, cs], xt_[:, kp, :, :],
                                    perf_mode=DR,
                                    start=(i == 0), stop=(i == n - 1),
                                )
                                i += 1
                        raw.append(ps)

                    # --- v: projected directly transposed into [t, d] ---
                    psv = psV.tile([P, 4, D], F32, name="psv")
                    n = 3 * KP
                    i = 0
                    for _, _, wv_t, xt_ in terms:
                        for kp in range(KP):
                            for j in range(4):
                                jts = slice(j * P, (j + 1) * P)
                                nc.tensor.matmul(
                                    psv[:, j, :], xt_[:, kp, :, jts],
                                    wv_t[:, kp, :, :],
                                    perf_mode=DR,
                                    start=(i == 0), stop=(i == n - 1),
                                )
                            i += 1
                    for j in range(4):
                        st = 4 * tr + j
                        nc.vector.tensor_copy(vp[:, st // 2, st % 2, :],
                                              psv[:, j, :])
                        if tr == 0:
                            nc.gpsimd.tensor_copy(v0b[:, st, :], psv[:, j, :])

                    # --- B: RMS norm + RoPE for q0, q1, k (bf16) ---
                    for s in range(3):
                        w_sb = qw_sb if s < 2 else kw_sb
                        src = raw[s]
                        sq = tmpp.tile([P, 512], BF16, name="sq")
                        nc.scalar.activation(sq[:], src[:], ACT_SQUARE)
                        ssum = psB.tile([1, 512], F32, name="ssum")
                        nc.tensor.matmul(ssum[:], ones_b[:], sq[:],
                                         start=True, stop=True)
                        # src holds 64*q: 1/sqrt(ssum/D + 64^2 eps) = rinv/64
                        rstd = tmpp.tile([1, 512], F32, name="rstd")
                        nc.scalar.activation(rstd[:], ssum[:], ACT_SQRT,
                                             scale=1.0 / D, bias=eps_sb[:])
                        rinv = tmpp.tile([1, 512], F32, name="rinv")
                        nc.vector.reciprocal_approx_fast(rinv[:], rstd[:])
                        rb = tmpp.tile([P, 512], F32, name="rb")
                        nc.gpsimd.partition_broadcast(rb[:], rinv[:])
                        nq = tmpp.tile([P, 512], BF16, name="nq")
                        nc.vector.scalar_tensor_tensor(
                            nq[:], src[:], w_sb[:], rb[:], MULT, MULT,
                        )
                        # RoPE: sin pre-rolled by 64 partitions with the
                        # rotate-half sign folded in; one full-width add.
                        psn = tmpp.tile([P, 512], BF16, name="psn")
                        nc.vector.tensor_mul(psn[0:H, :], nq[H:D, :],
                                             sin_sb[H:D, ts])
                        nc.vector.tensor_mul(psn[H:D, :], nq[0:H, :],
                                             sin_sb[0:H, ts])
                        pc = tmpp.tile([P, 512], BF16, name="pc")
                        nc.vector.tensor_mul(pc[:], nq[:], cos_sb[:, ts])
                        nc.vector.tensor_add(qT[s][:, ts], pc[:], psn[:])

            # ===== Phase C: attention + o_proj =====
            with (
                tc.tile_pool(name="atp", bufs=5) as atp,
                tc.tile_pool(name="adp", bufs=3) as adp,
                tc.tile_pool(name="cfp", bufs=2) as cfp,
                tc.tile_pool(name="otp", bufs=4) as otp,
                tc.tile_pool(name="attp", bufs=4) as attp,
                tc.tile_pool(name="psP", bufs=3, space="PSUM") as psP,
                tc.tile_pool(name="psCX", bufs=1, space="PSUM") as psCX,
                tc.tile_pool(name="psSM", bufs=1, space="PSUM") as psSM,
                tc.tile_pool(name="psD", bufs=3, space="PSUM") as psD,
            ):
                kT = qT[2]
                pending = []

                def make_task(qr, tt, nr, idx):
                    abs_tt = 4 * qr + tt
                    tts = slice(abs_tt * P, (abs_tt + 1) * P)
                    ns = slice(nr * 512, (nr + 1) * 512)

                    def go():
                        ps = psD.tile([P, 512], F32, name="psD_t")
                        for i, (cs, ws) in enumerate(
                                ((ctxC, woh_sb), (ctxL, woh_sb),
                                 (ctxC, wol_sb))):
                            nc.tensor.matmul(
                                ps[:], cs[:, :, tts], ws[:, :, ns],
                                perf_mode=DR,
                                start=(i == 0), stop=(i == 2))
                        ot = otp.tile([P, 512], BF16, name="ot")
                        r = idx % 8
                        if r in (0, 3, 6):
                            nc.vector.tensor_copy(ot[:], ps[:])
                        elif r in (1, 4, 7):
                            nc.scalar.copy(ot[:], ps[:])
                        else:
                            nc.gpsimd.tensor_copy(ot[:], ps[:])
                        if r % 2 == 0:
                            nc.sync.dma_start(out[tts, ns], ot[:])
                        else:
                            nc.scalar.dma_start(out[tts, ns], ot[:])
                    return go

                def emit_fill(k):
                    for _ in range(min(k, len(pending))):
                        pending.pop(0)()

                for qi, qr in enumerate(QR_ORDER):
                    qs = slice(qr * 512, (qr + 1) * 512)
                    at_dt = BF16 if qr == 0 else F8
                    for h in range(HQ):
                        n_off = 2 * qr
                        ctx_ps = psCX.tile([P, 512], F32, name="ctx_ps")
                        sums_ps = psSM.tile([1, 512], F32, name="sums_ps")
                        # --- fully-causal pairs below the diagonal block ---
                        for pi in range(n_off):
                            at = atp.tile([P, 2, 512], at_dt, name="at")
                            for half in range(2):
                                st = 2 * pi + half
                                s_ps = psP.tile([P, 512], F32, name="s_t")
                                nc.tensor.matmul(
                                    s_ps[:], kT[:, st * P:(st + 1) * P],
                                    qT[h][:, qs], start=True, stop=True)
                                nc.scalar.activation(
                                    at[:, half, :], s_ps[:], ACT_EXP,
                                    scale=SCALE, bias=shift_sb[:])
                            nc.tensor.matmul(
                                ctx_ps[:], vp[:, pi, :, :], at[:],
                                perf_mode=DR,
                                start=(pi == 0), stop=False)
                            nc.tensor.matmul(
                                sums_ps[:], w1_8[:, :, 0:1], at[:],
                                perf_mode=DR,
                                start=(pi == 0), stop=False)
                            emit_fill(1)
                        # --- diagonal 512x512 block, 128-query granular ---
                        for j in range(4):
                            jsl = slice(j * P, (j + 1) * P)
                            qjs = slice(qr * 512 + j * P,
                                        qr * 512 + (j + 1) * P)
                            sd = psP.tile([P, 4, P], F32, name="s_t")
                            for i in range(j + 1):
                                st = 4 * qr + i
                                nc.tensor.matmul(
                                    sd[:, i, :], kT[:, st * P:(st + 1) * P],
                                    qT[h][:, qjs], start=True, stop=True)
                            ad = adp.tile([P, 4, P], at_dt, name="ad")
                            nc.scalar.activation(
                                ad[:, 0:j + 1, :], sd[:, 0:j + 1, :],
                                ACT_EXP, scale=SCALE, bias=shift_sb[:])
                            # only the true-diagonal tile needs masking
                            nc.vector.tensor_tensor(
                                ad[:, j, :], ad[:, j, :], masks_sb[:], MIN)
                            if qr == 0:
                                for i in range(j + 1):
                                    nc.tensor.matmul(
                                        ctx_ps[:, jsl], v0b[:, i, :],
                                        ad[:, i, :],
                                        start=(i == 0), stop=(i == j))
                                    nc.tensor.matmul(
                                        sums_ps[0:1, jsl], ones_b[:],
                                        ad[:, i, :],
                                        start=(i == 0), stop=(i == j))
                            else:
                                np_full = (j + 1) // 2
                                for p_ in range(np_full):
                                    last = (j % 2 == 1) and (p_ == np_full - 1)
                                    nc.tensor.matmul(
                                        ctx_ps[:, jsl],
                                        vp[:, 2 * qr + p_, :, :],
                                        ad[:, 2 * p_:2 * p_ + 2, :],
                                        perf_mode=DR,
                                        start=False, stop=last)
                                    nc.tensor.matmul(
                                        sums_ps[0:1, jsl], w1_8[:, :, 0:1],
                                        ad[:, 2 * p_:2 * p_ + 2, :],
                                        perf_mode=DR,
                                        start=False, stop=last)
                                if j % 2 == 0:  # odd plane count: tail tile
                                    nc.tensor.matmul(
                                        ctx_ps[:, jsl],
                                        vp[:, 2 * qr + j // 2, j % 2, :],
                                        ad[:, j, :],
                                        start=False, stop=True)
                                    nc.tensor.matmul(
                                        sums_ps[0:1, jsl], w1_8[:, 0, 0:1],
                                        ad[:, j, :],
                                        start=False, stop=True)
                            emit_fill(1)
                        # --- normalize + fp8 hi/lo split of ctx ---
                        recip = attp.tile([1, 512], F32, name="recip")
                        nc.vector.reciprocal_approx_fast(recip[:], sums_ps[:])
                        rb = attp.tile([P, 512], F32, name="rbc")
                        nc.gpsimd.partition_broadcast(rb[:], recip[:])
                        cf = cfp.tile([P, 512], F32, name="cf")
                        nc.vector.tensor_mul(cf[:], ctx_ps[:], rb[:])
                        nc.gpsimd.tensor_copy(ctxC[:, h, qs], cf[:])
                        nc.vector.scalar_tensor_tensor(
                            ctxL[:, h, qs], cf[:], 1.0, ctxC[:, h, qs],
                            MULT, SUB)
                        emit_fill(1)
                    # queue this block's o_proj tiles as PE filler
                    for tt in range(4):
                        for nr in range(4):
                            pending.append(
                                make_task(qr, tt, nr, 4 * tt + nr))
                    if qi == len(QR_ORDER) - 1:
                        emit_fill(len(pending))

    nc.compile()
    return nc


_NC_CACHE = None


def get_nc():
    global _NC_CACHE
    if _NC_CACHE is None:
        _NC_CACHE = build_nc()
    return _NC_CACHE


F8NP = ml_dtypes.float8_e4m3
BF16NP = ml_dtypes.bfloat16


def _fold_hid(a):
    """[HID, C] -> [P, KP, 2, C] with hid = kp*256 + pl*128 + p."""
    c = a.shape[1]
    return np.ascontiguousarray(
        a.reshape(KP, 2, P, c).transpose(2, 0, 1, 3))


def _split8(a):
    hi = a.astype(F8NP)
    lo = (a - hi.astype(np.float32)).astype(F8NP)
    return hi, lo


def make_in_maps(x, cos, sin, Wq, Wk, Wv, Wo, q_norm_w, k_norm_w):
    x = np.asarray(x, dtype=np.float32).reshape(T, HID)
    xf = _fold_hid(np.ascontiguousarray(x.T).reshape(HID, T))
    xh, xl = _split8(xf)
    cosb = np.ascontiguousarray(
        np.asarray(cos, np.float32).T).astype(BF16NP)
    # rolled by 64 with rotate-half signs folded in:
    # psn[0:64] (subtracted in ref) uses rows 64:128 -> negate those rows
    sr = np.roll(np.asarray(sin, np.float32).T, 64, axis=0)
    sr[64:, :] *= -1.0
    sinb = np.ascontiguousarray(sr).astype(BF16NP)
    qwa = np.ascontiguousarray(
        np.asarray(q_norm_w, np.float32).reshape(D, 1))
    kwa = np.ascontiguousarray(
        np.asarray(k_norm_w, np.float32).reshape(D, 1))
    si = np.arange(P)[:, None]
    qi = np.arange(P)[None, :]
    masks = np.where(si <= qi, 240.0, 0.0).astype(BF16NP)
    Wq = np.asarray(Wq, np.float32) * WS
    Wk = np.asarray(Wk, np.float32) * WS
    Wv = np.asarray(Wv, np.float32) * VS
    Wo = np.asarray(Wo, np.float32) * BETA
    in_maps = []
    for c in range(N_CORES):
        wqh_, wql_ = _split8(_fold_hid(Wq[:, c * HQ * D:(c + 1) * HQ * D]))
        wkh_, wkl_ = _split8(_fold_hid(Wk[:, c * D:(c + 1) * D]))
        wvh_, wvl_ = _split8(_fold_hid(Wv[:, c * D:(c + 1) * D]))
        wo_ = np.ascontiguousarray(
            Wo[c * HQ * D:(c + 1) * HQ * D, :].reshape(HQ, P, HID)
            .transpose(1, 0, 2))
        woh_, wol_ = _split8(wo_)
        in_maps.append({
            "xh": xh, "xl": xl,
            "wqh": wqh_, "wql": wql_,
            "wkh": wkh_, "wkl": wkl_,
            "wvh": wvh_, "wvl": wvl_,
            "woh": woh_, "wol": wol_,
            "cosT": cosb, "sinT": sinb,
            "qw": qwa, "kw": kwa,
            "masks": masks,
        })
    return in_maps


def kernel(x, cos, sin, Wq, Wk, Wv, Wo, q_norm_w, k_norm_w):
    nc = get_nc()
    in_maps = make_in_maps(x, cos, sin, Wq, Wk, Wv, Wo, q_norm_w, k_norm_w)
    res = run_bass_kernel_spmd(nc, in_maps, core_ids=list(range(N_CORES)))
    acc = np.zeros((T, HID), dtype=np.float32)
    for c in range(N_CORES):
        acc += np.asarray(res.results[c]["out"], np.float32)
    acc *= 1.0 / (VS * BETA)
    return acc.reshape(1, T, HID)


# revision 11
# speedup vs baseline: 1.0656x; 1.0656x over previous
"""GQA attention block (B=1, T=2048, HID=2048, NQ=16, NKV=8, D=128) on 8 TRN2
NeuronCores.

Sharding: tensor-parallel over heads. Core c owns q-heads {2c, 2c+1} and
kv-head c. The 8 partial outputs are summed on the host (scaled 1/(VS*BETA)).

v2 speed strategy (validated against the TimelineSim cost model + f64 ref):
  - projections: 3-term split-fp8 (xh*wh + xl*wh + xh*wl) with K=256
    DoubleRow matmuls (0.5 cyc/row in the cost model). Wq/Wk pre-scaled by
    WS=64 (cancels through RMS norm), Wv by VS=32.
  - V is projected directly transposed ([t, d] tiles, stationary = x chunk)
    so no PE transposes / identity are needed.
  - q/k: RMS-norm + RoPE on DVE in bf16; Act reads raw projections straight
    from PSUM (no staging copy).
  - attention: at = exp(score/sqrt(D) - 2) fp8 for q-rows >= 512, bf16 for
    the first 512 rows. Causal handling is fine-grained on the diagonal
    512x512 block: per 128-query subtile only the needed key tiles are
    computed, and only the true-diagonal 128x128 tile is min-masked
    (mask in {0, 240}: min(sat, 0) = 0 kills acausal fp8-overflowed exp).
  - denominators: ones-stationary matmuls accumulated alongside ctx (PSUM).
  - o_proj: 3-term split-fp8 DoubleRow with BOTH heads packed into K=256
    (ctx split hi/lo on DVE/Pool; Wo pre-scaled by BETA=64, ctx carries
    VS=32; host divides by 2048). Output rows for q-blocks {0,2,3} are
    DMA'd f32 straight from PSUM (no copy); the last-processed block (1)
    goes through bf16 copies for a short tail.
  - schedule: q-blocks processed in order 0,3,2,1; each block's o_proj
    tiles are interleaved as PE filler into the next block's attention
    (which is Act-exp paced), keeping the PE queue dense.
"""

import sys

sys.path.insert(0, "/opt/trn_rl_repo")

import numpy as np
import ml_dtypes

import concourse.bass as bass  # noqa: F401  (bass must import before tile)
import concourse.mybir as mybir
import concourse.tile as tile
from concourse import bacc
from concourse.bass_utils import run_bass_kernel_spmd

N_CORES = 8
T = 2048
HID = 2048
NQ, NKV, D = 16, 8, 128
HQ = NQ // N_CORES  # q heads per core = 2
EPS = 1e-6
SCALE = D**-0.5
SHIFT = 2.0
WS = 64.0   # Wq/Wk pre-scale (cancels in RMS norm)
VS = 32.0   # Wv pre-scale == ctx scale alpha (fp8 range)
BETA = 64.0  # Wo pre-scale (fp8 range); host divides by VS*BETA

P = 128
H = D // 2
KP = HID // 256     # 8 K-pair chunks of 256
NTR = T // 512      # 4 T-ranges of 512

F32 = mybir.dt.float32
BF16 = mybir.dt.bfloat16
F8 = mybir.dt.float8e4
DR = mybir.MatmulPerfMode.DoubleRow
ACT_EXP = mybir.ActivationFunctionType.Exp
ACT_SQRT = mybir.ActivationFunctionType.Sqrt
ACT_SQUARE = mybir.ActivationFunctionType.Square
MIN = mybir.AluOpType.min
MULT = mybir.AluOpType.mult
SUB = mybir.AluOpType.subtract

QR_ORDER = [0, 3, 2, 1]  # last one takes the bf16-copy output path


def build_nc():
    nc = bacc.Bacc("TRN2", target_bir_lowering=False, debug=False,
                   num_devices=N_CORES)

    # ---- DRAM tensors (names = in_map keys) ----
    xh = nc.dram_tensor("xh", [P, KP, 2, T], F8, kind="ExternalInput")
    xl = nc.dram_tensor("xl", [P, KP, 2, T], F8, kind="ExternalInput")
    wqh = nc.dram_tensor("wqh", [P, KP, 2, HQ * D], F8, kind="ExternalInput")
    wql = nc.dram_tensor("wql", [P, KP, 2, HQ * D], F8, kind="ExternalInput")
    wkh = nc.dram_tensor("wkh", [P, KP, 2, D], F8, kind="ExternalInput")
    wkl = nc.dram_tensor("wkl", [P, KP, 2, D], F8, kind="ExternalInput")
    wvh = nc.dram_tensor("wvh", [P, KP, 2, D], F8, kind="ExternalInput")
    wvl = nc.dram_tensor("wvl", [P, KP, 2, D], F8, kind="ExternalInput")
    woh = nc.dram_tensor("woh", [P, HQ, HID], F8, kind="ExternalInput")
    wol = nc.dram_tensor("wol", [P, HQ, HID], F8, kind="ExternalInput")
    cosT = nc.dram_tensor("cosT", [P, T], BF16, kind="ExternalInput")
    sinT = nc.dram_tensor("sinT", [P, T], BF16, kind="ExternalInput")
    qw = nc.dram_tensor("qw", [P, 1], F32, kind="ExternalInput")
    kw = nc.dram_tensor("kw", [P, 1], F32, kind="ExternalInput")
    masks = nc.dram_tensor("masks", [P, P], BF16, kind="ExternalInput")
    out = nc.dram_tensor("out", [T, HID], BF16, kind="ExternalOutput")

    with tile.TileContext(nc) as tc:
        with (
            tc.tile_pool(name="cst", bufs=1) as cst,
            tc.tile_pool(name="fin", bufs=1) as fin,
        ):
            # ---------- constants / weights resident in SBUF ----------
            wqh_sb = cst.tile([P, KP, 2, HQ * D], F8)
            wql_sb = cst.tile([P, KP, 2, HQ * D], F8)
            wkh_sb = cst.tile([P, KP, 2, D], F8)
            wkl_sb = cst.tile([P, KP, 2, D], F8)
            wvh_sb = cst.tile([P, KP, 2, D], F8)
            wvl_sb = cst.tile([P, KP, 2, D], F8)
            woh_sb = cst.tile([P, HQ, HID], F8)
            wol_sb = cst.tile([P, HQ, HID], F8)
            masks_sb = cst.tile([P, P], BF16)
            cos_sb = cst.tile([P, T], BF16)
            sin_sb = cst.tile([P, T], BF16)
            qw_sb = cst.tile([P, 1], F32)
            kw_sb = cst.tile([P, 1], F32)
            nc.gpsimd.dma_start(qw_sb[:], qw[:])
            nc.gpsimd.dma_start(kw_sb[:], kw[:])
            ones_b = cst.tile([P, 1], BF16)
            nc.vector.memset(ones_b[:], 1.0)
            # DoubleRow ldweights requires the 2-plane dim step % 16 == 0
            w1_8 = cst.tile([P, 2, 16], F8)
            nc.vector.memset(w1_8[:], 1.0)
            eps_sb = cst.tile([1, 1], F32)
            nc.vector.memset(eps_sb[:], EPS * WS * WS)
            shift_sb = cst.tile([P, 1], F32)
            nc.vector.memset(shift_sb[:], -SHIFT)

            # post RMS+RoPE q/k in bf16 (d on partitions)
            qT = [fin.tile([P, T], BF16, name=f"qT_{s}") for s in range(3)]
            # V (VS x): fp8 plane-pairs (plane = st parity) + bf16 st 0-3
            vp = fin.tile([P, T // 256, 2, D], F8)
            v0b = fin.tile([P, 4, D], BF16)
            # normalized ctx (VS x), fp8 hi/lo, plane = head
            ctxC = fin.tile([P, HQ, T], F8)
            ctxL = fin.tile([P, HQ, T], F8)

            # ==== Phase A (split-fp8 DR projections) + B (RMS+RoPE) ====
            with (
                tc.tile_pool(name="xp", bufs=4) as xp,
                tc.tile_pool(name="tmpp", bufs=6) as tmpp,
                tc.tile_pool(name="psA", bufs=4, space="PSUM") as psA,
                tc.tile_pool(name="psV", bufs=2, space="PSUM") as psV,
                tc.tile_pool(name="psB", bufs=2, space="PSUM") as psB,
            ):
                for tr in range(NTR):
                    ts = slice(tr * 512, (tr + 1) * 512)
                    xhc = xp.tile([P, KP, 2, 512], F8, name="xhc")
                    xlc = xp.tile([P, KP, 2, 512], F8, name="xlc")
                    if tr == 0:
                        # ordered for fastest PE start: wq-hi + x-hi first,
                        # spread across SP/Act/DVE queues (SEQ serializes
                        # per queue, transfers serialize on DMA_ENGINES)
                        nc.sync.dma_start(wqh_sb[:], wqh[:])
                        nc.scalar.dma_start(xhc[:, 0:4, :, :], xh[:, 0:4, :, ts])
                        nc.sync.dma_start(wkh_sb[:], wkh[:])
                        nc.sync.dma_start(wvh_sb[:], wvh[:])
                        nc.scalar.dma_start(xhc[:, 4:8, :, :], xh[:, 4:8, :, ts])
                        nc.scalar.dma_start(xlc[:, 0:4, :, :], xl[:, 0:4, :, ts])
                        nc.scalar.dma_start(xlc[:, 4:8, :, :], xl[:, 4:8, :, ts])
                        nc.gpsimd.dma_start(wql_sb[:], wql[:])
                        nc.gpsimd.dma_start(wkl_sb[:], wkl[:])
                        nc.gpsimd.dma_start(wvl_sb[:], wvl[:])
                        nc.scalar.dma_start(cos_sb[:], cosT[:])
                        nc.scalar.dma_start(sin_sb[:], sinT[:])
                        nc.gpsimd.dma_start(masks_sb[:], masks[:])
                    else:
                        nc.sync.dma_start(xhc[:], xh[:, :, :, ts])
                        nc.sync.dma_start(xlc[:], xl[:, :, :, ts])
                    if tr == 1:
                        nc.gpsimd.dma_start(woh_sb[:], woh[:])
                        nc.gpsimd.dma_start(wol_sb[:], wol[:])

                    terms = ((wqh_sb, wkh_sb, wvh_sb, xhc),
                             (wqh_sb, wkh_sb, wvh_sb, xlc),
                             (wql_sb, wkl_sb, wvl_sb, xhc))

                    # --- projections q0, q1, k (d on partitions) ---
                    raw = []
                    for s in range(3):
                        ps = psA.tile([P, 512], F32, name="psA_t")
                        n = 3 * KP
                        i = 0
                        for wq_t, wk_t, _, xt_ in terms:
                            wt = wq_t if s < 2 else wk_t
                            cs = slice(s * D, (s + 1) * D) if s < 2 \
                                else slice(0, D)
                            for kp in range(KP):
                                nc.tensor.matmul(
                                    ps[:], wt[:, kp, :, cs], xt_[:, kp, :, :],
                                    perf_mode=DR,
                                    start=(i == 0), stop=(i == n - 1),
                                )
                                i += 1
                        raw.append(ps)

                    # --- v: projected directly transposed into [t, d] ---
                    psv = psV.tile([P, 4, D], F32, name="psv")
                    n = 3 * KP
                    i = 0
                    for _, _, wv_t, xt_ in terms:
                        for kp in range(KP):
                            for j in range(4):
                                jts = slice(j * P, (j + 1) * P)
                                nc.tensor.matmul(
                                    psv[:, j, :], xt_[:, kp, :, jts],
                                    wv_t[:, kp, :, :],
                                    perf_mode=DR,
                                    start=(i == 0), stop=(i == n - 1),
                                )
                            i += 1
                    for j in range(4):
                        st = 4 * tr + j
                        nc.vector.tensor_copy(vp[:, st // 2, st % 2, :],
                                              psv[:, j, :])
                        if tr == 0:
                            nc.gpsimd.tensor_copy(v0b[:, st, :], psv[:, j, :])

                    # --- B: RMS norm + RoPE for q0, q1, k (bf16) ---
                    for s in range(3):
                        w_sb = qw_sb if s < 2 else kw_sb
                        src = raw[s]
                        sq = tmpp.tile([P, 512], BF16, name="sq")
                        nc.scalar.activation(sq[:], src[:], ACT_SQUARE)
                        ssum = psB.tile([1, 512], F32, name="ssum")
                        nc.tensor.matmul(ssum[:], ones_b[:], sq[:],
                                         start=True, stop=True)
                        # src holds 64*q: 1/sqrt(ssum/D + 64^2 eps) = rinv/64
                        rstd = tmpp.tile([1, 512], F32, name="rstd")
                        nc.scalar.activation(rstd[:], ssum[:], ACT_SQRT,
                                             scale=1.0 / D, bias=eps_sb[:])
                        rinv = tmpp.tile([1, 512], F32, name="rinv")
                        nc.vector.reciprocal_approx_fast(rinv[:], rstd[:])
                        rb = tmpp.tile([P, 512], F32, name="rb")
                        nc.gpsimd.partition_broadcast(rb[:], rinv[:])
                        nq = tmpp.tile([P, 512], BF16, name="nq")
                        nc.vector.scalar_tensor_tensor(
                            nq[:], src[:], w_sb[:], rb[:], MULT, MULT,
                        )
                        # RoPE: sin pre-rolled by 64 partitions with the
                        # rotate-half sign folded in; one full-width add.
                        psn = tmpp.tile([P, 512], BF16, name="psn")
                        nc.vector.tensor_mul(psn[0:H, :], nq[H:D, :],
                                             sin_sb[H:D, ts])
                        nc.vector.tensor_mul(psn[H:D, :], nq[0:H, :],
                                             sin_sb[0:H, ts])
                        pc = tmpp.tile([P, 512], BF16, name="pc")
                        nc.vector.tensor_mul(pc[:], nq[:], cos_sb[:, ts])
                        nc.vector.tensor_add(qT[s][:, ts], pc[:], psn[:])

            # ===== Phase C: attention + o_proj =====
            with (
                tc.tile_pool(name="atp", bufs=5) as atp,
                tc.tile_pool(name="adp", bufs=3) as adp,
                tc.tile_pool(name="cfp", bufs=2) as cfp,
                tc.tile_pool(name="otp", bufs=4) as otp,
                tc.tile_pool(name="attp", bufs=4) as attp,
                tc.tile_pool(name="psP", bufs=3, space="PSUM") as psP,
                tc.tile_pool(name="psCX", bufs=1, space="PSUM") as psCX,
                tc.tile_pool(name="psSM", bufs=1, space="PSUM") as psSM,
                tc.tile_pool(name="psD", bufs=3, space="PSUM") as psD,
            ):
                kT = qT[2]
                pending = []

                def make_task(qr, tt, nr, idx):
                    abs_tt = 4 * qr + tt
                    tts = slice(abs_tt * P, (abs_tt + 1) * P)
                    ns = slice(nr * 512, (nr + 1) * 512)

                    def go():
                        ps = psD.tile([P, 512], F32, name="psD_t")
                        for i, (cs, ws) in enumerate(
                                ((ctxC, woh_sb), (ctxL, woh_sb),
                                 (ctxC, wol_sb))):
                            nc.tensor.matmul(
                                ps[:], cs[:, :, tts], ws[:, :, ns],
                                perf_mode=DR,
                                start=(i == 0), stop=(i == 2))
                        ot = otp.tile([P, 512], BF16, name="ot")
                        r = idx % 3
                        if r == 0:
                            nc.vector.tensor_copy(ot[:], ps[:])
                        elif r == 1:
                            nc.scalar.copy(ot[:], ps[:])
                        else:
                            nc.gpsimd.tensor_copy(ot[:], ps[:])
                        # out DMAs only on sync: a dma_start blocks its
                        # issuing engine's SEQ until the copy dependency
                        # resolves, so compute queues must not carry them
                        nc.sync.dma_start(out[tts, ns], ot[:])
                    return go

                def emit_fill(k):
                    for _ in range(min(k, len(pending))):
                        pending.pop(0)()

                for qi, qr in enumerate(QR_ORDER):
                    qs = slice(qr * 512, (qr + 1) * 512)
                    at_dt = BF16 if qr == 0 else F8
                    for h in range(HQ):
                        n_off = 2 * qr
                        ctx_ps = psCX.tile([P, 512], F32, name="ctx_ps")
                        sums_ps = psSM.tile([1, 512], F32, name="sums_ps")
                        # --- fully-causal pairs below the diagonal block ---
                        for pi in range(n_off):
                            at = atp.tile([P, 2, 512], at_dt, name="at")
                            for half in range(2):
                                st = 2 * pi + half
                                s_ps = psP.tile([P, 512], F32, name="s_t")
                                nc.tensor.matmul(
                                    s_ps[:], kT[:, st * P:(st + 1) * P],
                                    qT[h][:, qs], start=True, stop=True)
                                nc.scalar.activation(
                                    at[:, half, :], s_ps[:], ACT_EXP,
                                    scale=SCALE, bias=shift_sb[:])
                            nc.tensor.matmul(
                                ctx_ps[:], vp[:, pi, :, :], at[:],
                                perf_mode=DR,
                                start=(pi == 0), stop=False)
                            nc.tensor.matmul(
                                sums_ps[:], w1_8[:, :, 0:1], at[:],
                                perf_mode=DR,
                                start=(pi == 0), stop=False)
                            emit_fill(1)
                        # --- diagonal 512x512 block, 128-query granular ---
                        for j in range(4):
                            jsl = slice(j * P, (j + 1) * P)
                            qjs = slice(qr * 512 + j * P,
                                        qr * 512 + (j + 1) * P)
                            sd = psP.tile([P, 4, P], F32, name="s_t")
                            for i in range(j + 1):
                                st = 4 * qr + i
                                nc.tensor.matmul(
                                    sd[:, i, :], kT[:, st * P:(st + 1) * P],
                                    qT[h][:, qjs], start=True, stop=True)
                            ad = adp.tile([P, 4, P], at_dt, name="ad")
                            nc.scalar.activation(
                                ad[:, 0:j + 1, :], sd[:, 0:j + 1, :],
                                ACT_EXP, scale=SCALE, bias=shift_sb[:])
                            # only the true-diagonal tile needs masking
                            nc.vector.tensor_tensor(
                                ad[:, j, :], ad[:, j, :], masks_sb[:], MIN)
                            if qr == 0:
                                for i in range(j + 1):
                                    nc.tensor.matmul(
                                        ctx_ps[:, jsl], v0b[:, i, :],
                                        ad[:, i, :],
                                        start=(i == 0), stop=(i == j))
                                    nc.tensor.matmul(
                                        sums_ps[0:1, jsl], ones_b[:],
                                        ad[:, i, :],
                                        start=(i == 0), stop=(i == j))
                            else:
                                np_full = (j + 1) // 2
                                for p_ in range(np_full):
                                    last = (j % 2 == 1) and (p_ == np_full - 1)
                                    nc.tensor.matmul(
                                        ctx_ps[:, jsl],
                                        vp[:, 2 * qr + p_, :, :],
                                        ad[:, 2 * p_:2 * p_ + 2, :],
                                        perf_mode=DR,
                                        start=False, stop=last)
                                    nc.tensor.matmul(
                                        sums_ps[0:1, jsl], w1_8[:, :, 0:1],
                                        ad[:, 2 * p_:2 * p_ + 2, :],
                                        perf_mode=DR,
                                        start=False, stop=last)
                                if j % 2 == 0:  # odd plane count: tail tile
                                    nc.tensor.matmul(
                                        ctx_ps[:, jsl],
                                        vp[:, 2 * qr + j // 2, j % 2, :],
                                        ad[:, j, :],
                                        start=False, stop=True)
                                    nc.tensor.matmul(
                                        sums_ps[0:1, jsl], w1_8[:, 0, 0:1],
                                        ad[:, j, :],
                                        start=False, stop=True)
                            emit_fill(1)
                        # --- normalize + fp8 hi/lo split of ctx ---
                        recip = attp.tile([1, 512], F32, name="recip")
                        nc.vector.reciprocal_approx_fast(recip[:], sums_ps[:])
                        rb = attp.tile([P, 512], F32, name="rbc")
                        nc.gpsimd.partition_broadcast(rb[:], recip[:])
                        cf = cfp.tile([P, 512], F32, name="cf")
                        nc.vector.tensor_mul(cf[:], ctx_ps[:], rb[:])
                        nc.gpsimd.tensor_copy(ctxC[:, h, qs], cf[:])
                        nc.vector.scalar_tensor_tensor(
                            ctxL[:, h, qs], cf[:], 1.0, ctxC[:, h, qs],
                            MULT, SUB)
                        emit_fill(1)
                    # queue this block's o_proj tiles as PE filler
                    for tt in range(4):
                        for nr in range(4):
                            pending.append(
                                make_task(qr, tt, nr, 4 * tt + nr))
                    if qi == len(QR_ORDER) - 1:
                        emit_fill(len(pending))

    nc.compile()
    return nc


_NC_CACHE = None


def get_nc():
    global _NC_CACHE
    if _NC_CACHE is None:
        _NC_CACHE = build_nc()
    return _NC_CACHE


F8NP = ml_dtypes.float8_e4m3
BF16NP = ml_dtypes.bfloat16


def _fold_hid(a):
    """[HID, C] -> [P, KP, 2, C] with hid = kp*256 + pl*128 + p."""
    c = a.shape[1]
    return np.ascontiguousarray(
        a.reshape(KP, 2, P, c).transpose(2, 0, 1, 3))


def _split8(a):
    hi = a.astype(F8NP)
    lo = (a - hi.astype(np.float32)).astype(F8NP)
    return hi, lo


def make_in_maps(x, cos, sin, Wq, Wk, Wv, Wo, q_norm_w, k_norm_w):
    x = np.asarray(x, dtype=np.float32).reshape(T, HID)
    xf = _fold_hid(np.ascontiguousarray(x.T).reshape(HID, T))
    xh, xl = _split8(xf)
    cosb = np.ascontiguousarray(
        np.asarray(cos, np.float32).T).astype(BF16NP)
    # rolled by 64 with rotate-half signs folded in:
    # psn[0:64] (subtracted in ref) uses rows 64:128 -> negate those rows
    sr = np.roll(np.asarray(sin, np.float32).T, 64, axis=0)
    sr[64:, :] *= -1.0
    sinb = np.ascontiguousarray(sr).astype(BF16NP)
    qwa = np.ascontiguousarray(
        np.asarray(q_norm_w, np.float32).reshape(D, 1))
    kwa = np.ascontiguousarray(
        np.asarray(k_norm_w, np.float32).reshape(D, 1))
    si = np.arange(P)[:, None]
    qi = np.arange(P)[None, :]
    masks = np.where(si <= qi, 240.0, 0.0).astype(BF16NP)
    Wq = np.asarray(Wq, np.float32) * WS
    Wk = np.asarray(Wk, np.float32) * WS
    Wv = np.asarray(Wv, np.float32) * VS
    Wo = np.asarray(Wo, np.float32) * BETA
    in_maps = []
    for c in range(N_CORES):
        wqh_, wql_ = _split8(_fold_hid(Wq[:, c * HQ * D:(c + 1) * HQ * D]))
        wkh_, wkl_ = _split8(_fold_hid(Wk[:, c * D:(c + 1) * D]))
        wvh_, wvl_ = _split8(_fold_hid(Wv[:, c * D:(c + 1) * D]))
        wo_ = np.ascontiguousarray(
            Wo[c * HQ * D:(c + 1) * HQ * D, :].reshape(HQ, P, HID)
            .transpose(1, 0, 2))
        woh_, wol_ = _split8(wo_)
        in_maps.append({
            "xh": xh, "xl": xl,
            "wqh": wqh_, "wql": wql_,
            "wkh": wkh_, "wkl": wkl_,
            "wvh": wvh_, "wvl": wvl_,
            "woh": woh_, "wol": wol_,
            "cosT": cosb, "sinT": sinb,
            "qw": qwa, "kw": kwa,
            "masks": masks,
        })
    return in_maps


def kernel(x, cos, sin, Wq, Wk, Wv, Wo, q_norm_w, k_norm_w):
    nc = get_nc()
    in_maps = make_in_maps(x, cos, sin, Wq, Wk, Wv, Wo, q_norm_w, k_norm_w)
    res = run_bass_kernel_spmd(nc, in_maps, core_ids=list(range(N_CORES)))
    acc = np.zeros((T, HID), dtype=np.float32)
    for c in range(N_CORES):
        acc += np.asarray(res.results[c]["out"], np.float32)
    acc *= 1.0 / (VS * BETA)
    return acc.reshape(1, T, HID)
